# revision 2
# baseline (speedup 1.0000x reference)
"""Trainium2 Bass kernel for nn_CrossAttention (B=4, Lq=1024, Lkv=2048, C=1024, H=16).

Sharding (8 cores): core c -> batch b = c//2, head-group g = c%2 (8 of 16 heads).
Per-core TP over heads: q/k/v weights column-sharded, proj row-sharded; each core
computes a partial (C x Lq) projection output; host sums the pair and adds bias.

Device pipeline per core (all matmuls bf16 with fp32 PSUM accumulation):
  qhT  = (q_w_g * D^-0.5 @ q^T)            [512, 1024]   (j_local, l)
  kT   = (kw_g @ kv^T)                     [512, 2048]   (j_local, t)
  v    = (kv @ vw_g^T)                     [2048, 520]   (t, 8*65) with ones cols
  per head: S^T[t,l] = kT_h^T-slices x qhT_h   (K=64, 2-head row-packed)
            S^T += attn_pos^T (DVE add, t<1024)
            E = exp(S^T) (ACT, no max-subtraction: logits are O(5))
            O'aug^T[65,l] = v_aug^T x E  (ones row 64 = softmax denom Z)
            x^T_h = O'^T[0:64] * (1/Z)   (partition-broadcast recip)
  outp[o,l] = pw_g^T x x^T  (partial, summed across the core pair on host)
"""

import sys
import os

for _p in ("/opt/trn_rl_repo",):
    if _p not in sys.path and os.path.isdir(_p):
        sys.path.append(_p)

import numpy as np
import ml_dtypes

import concourse.bass as bass
import concourse.bacc as bacc
import concourse.mybir as mybir
from concourse.tile import TileContext
from concourse.bass_utils import run_bass_kernel_spmd

BF16 = mybir.dt.bfloat16
F32 = mybir.dt.float32
AF = mybir.ActivationFunctionType
ALU = mybir.AluOpType

B, Lq, Lkv, C, H, D, Lpos = 4, 1024, 2048, 1024, 16, 64, 1024
HPC = 8            # heads per core
JC = HPC * D       # 512: local head-dim width
N_CORES = 8
NT = Lkv // 128    # 16 t-tiles
NPOS = Lpos // 128  # 8 t-tiles carrying attn_pos


def build_kernel():
    nc = bacc.Bacc(trn_type="TRN2")

    qT = nc.declare_dram_parameter("qT", [C, Lq], BF16, isOutput=False)
    kvT = nc.declare_dram_parameter("kvT", [C, Lkv], BF16, isOutput=False)
    qwT = nc.declare_dram_parameter("qwT", [C, JC], BF16, isOutput=False)
    kwT = nc.declare_dram_parameter("kwT", [C, JC], BF16, isOutput=False)
    vwT = nc.declare_dram_parameter("vwT", [C, JC], BF16, isOutput=False)
    posT = nc.declare_dram_parameter("posT", [HPC, Lpos, Lq], BF16, isOutput=False)
    pwT = nc.declare_dram_parameter("pwT", [JC, C], BF16, isOutput=False)
    outp = nc.declare_dram_parameter("outp", [C, Lq], F32, isOutput=True)

    with TileContext(nc) as tc:
        with (
            tc.tile_pool(name="persist", bufs=1) as persist,
            tc.tile_pool(name="stage", bufs=1) as stage,
            tc.tile_pool(name="pos", bufs=4) as pospool,
            tc.tile_pool(name="exps", bufs=4) as expool,
            tc.tile_pool(name="small", bufs=4) as small,
            tc.tile_pool(name="osb", bufs=3) as osb,
            tc.tile_pool(name="ps", bufs=2, space="PSUM") as ps,
            tc.tile_pool(name="pso", bufs=2, space="PSUM") as pso,
        ):
            # ---- stage inputs ----
            qT_sb = stage.tile([128, C // 128, Lq], BF16)       # 2 MB
            nc.sync.dma_start(out=qT_sb[:], in_=qT.rearrange("(cc p) l -> p cc l", p=128))
            kvT_sb = stage.tile([128, C // 128, Lkv], BF16)     # 4 MB
            nc.sync.dma_start(out=kvT_sb[:], in_=kvT.rearrange("(cc p) t -> p cc t", p=128))
            qwT_sb = stage.tile([128, C // 128, JC], BF16)      # 1 MB
            nc.sync.dma_start(out=qwT_sb[:], in_=qwT.rearrange("(cc p) j -> p cc j", p=128))
            kwT_sb = stage.tile([128, C // 128, JC], BF16)
            nc.sync.dma_start(out=kwT_sb[:], in_=kwT.rearrange("(cc p) j -> p cc j", p=128))
            vwT_sb = stage.tile([128, C // 128, JC], BF16)
            nc.sync.dma_start(out=vwT_sb[:], in_=vwT.rearrange("(cc p) j -> p cc j", p=128))
            pwT_sb = stage.tile([128, JC // 128, C], BF16)      # 1 MB
            nc.sync.dma_start(out=pwT_sb[:], in_=pwT.rearrange("(jc p) o -> p jc o", p=128))

            # ---- persistent intermediates ----
            qhT_sb = persist.tile([128, JC // 128, Lq], BF16)   # (j%128, j//128, l)
            kT_sb = persist.tile([128, JC // 128, Lkv], BF16)   # (j%128, j//128, t)
            v_sb = persist.tile([128, NT, HPC * 65], BF16)      # (t%128, t//128, h*65+d; col 64 = ones)
            xT_sb = persist.tile([128, JC // 128, Lq], BF16)    # (j%128, j//128, l)

            # ones columns of v_aug (softmax denominator accumulators)
            for h in range(HPC):
                nc.gpsimd.memset(v_sb[:, :, h * 65 + 64 : h * 65 + 65], 1.0)

            NC = C // 128  # 8 contraction chunks

            # ---- q projection: qhT[j, l] ----
            for jt in range(JC // 128):
                for lh in range(Lq // 512):
                    acc = ps.tile([128, 512], F32, tag="s")
                    for cc in range(NC):
                        nc.tensor.matmul(
                            acc[:],
                            lhsT=qwT_sb[:, cc, jt * 128 : (jt + 1) * 128],
                            rhs=qT_sb[:, cc, lh * 512 : (lh + 1) * 512],
                            start=(cc == 0),
                            stop=(cc == NC - 1),
                        )
                    nc.vector.tensor_copy(qhT_sb[:, jt, lh * 512 : (lh + 1) * 512], acc[:])

            # ---- k projection: kT[j, t] ---- (per head-pair jt)
            for jt in range(JC // 128):
                for tch in range(Lkv // 512):
                    acc = ps.tile([128, 512], F32, tag="s")
                    for cc in range(NC):
                        nc.tensor.matmul(
                            acc[:],
                            lhsT=kwT_sb[:, cc, jt * 128 : (jt + 1) * 128],
                            rhs=kvT_sb[:, cc, tch * 512 : (tch + 1) * 512],
                            start=(cc == 0),
                            stop=(cc == NC - 1),
                        )
                    nc.vector.tensor_copy(kT_sb[:, jt, tch * 512 : (tch + 1) * 512], acc[:])

            # ---- v projection: v[t, j] ----
            for tt in range(NT):
                acc = ps.tile([128, 512], F32, tag="s")
                for cc in range(NC):
                    nc.tensor.matmul(
                        acc[:],
                        lhsT=kvT_sb[:, cc, tt * 128 : (tt + 1) * 128],
                        rhs=vwT_sb[:, cc, :],
                        start=(cc == 0),
                        stop=(cc == NC - 1),
                    )
                nc.vector.tensor_copy(
                    v_sb[:, tt, :].rearrange("p (h c) -> p h c", c=65)[:, :, 0:64],
                    acc[:].rearrange("p (h c) -> p h c", c=64),
                )

            # ---- attention, one head-pair at a time ----
            for p in range(HPC // 2):
                po = [
                    pso.tile([65, Lq], F32, tag="o", name=f"po{p}_{i}")
                    for i in range(2)
                ]
                for tt in range(NT):
                    s_tiles = []
                    for sub in range(2):  # head 2p+sub; row-packed K=64 matmuls
                        st = ps.tile([128, Lq], F32, tag="s")
                        s_tiles.append(st)
                        lo, hi = sub * 64, sub * 64 + 64
                        for lh in range(Lq // 512):
                            nc.tensor.matmul(
                                st[:, lh * 512 : (lh + 1) * 512],
                                lhsT=kT_sb[lo:hi, p, tt * 128 : (tt + 1) * 128],
                                rhs=qhT_sb[lo:hi, p, lh * 512 : (lh + 1) * 512],
                                start=True,
                                stop=True,
                            )
                    for sub in range(2):
                        h = 2 * p + sub
                        st = s_tiles[sub]
                        if tt < NPOS:
                            pt = pospool.tile([128, Lq], BF16)
                            nc.sync.dma_start(
                                out=pt[:], in_=posT[h, tt * 128 : (tt + 1) * 128, :]
                            )
                            nc.vector.tensor_tensor(st[:], st[:], pt[:], op=ALU.add)
                        et = expool.tile([128, Lq], BF16)
                        nc.scalar.activation(et[:], st[:], AF.Exp)
                        for lh in range(Lq // 512):
                            nc.tensor.matmul(
                                po[sub][:, lh * 512 : (lh + 1) * 512],
                                lhsT=v_sb[:, tt, h * 65 : h * 65 + 65],
                                rhs=et[:, lh * 512 : (lh + 1) * 512],
                                start=(tt == 0),
                                stop=(tt == NT - 1),
                            )
                # normalize: x^T_h = O'[0:64] * 1/Z
                for sub in range(2):
                    r = small.tile([1, Lq], F32, tag="r")
                    nc.vector.reciprocal(r[:], po[sub][64:65, :])
                    r64 = small.tile([64, Lq], F32, tag="r64")
                    nc.gpsimd.partition_broadcast(r64[:], r[:])
                    if sub == 0:
                        nc.vector.tensor_tensor(
                            xT_sb[0:64, p, :], po[sub][0:64, :], r64[:], op=ALU.mult
                        )
                    else:
                        xt = small.tile([64, Lq], BF16, tag="xt")
                        nc.vector.tensor_tensor(
                            xt[:], po[sub][0:64, :], r64[:], op=ALU.mult
                        )
                        # partition shift 0..63 -> 64..127 via SBUF->SBUF DMA
                        nc.sync.dma_start(out=xT_sb[64:128, p, :], in_=xt[:])

            # ---- output projection (partial): outp[o, l] ----
            for ot in range(C // 128):
                ob = osb.tile([128, Lq], F32)
                for lh in range(Lq // 512):
                    acc = ps.tile([128, 512], F32, tag="s")
                    for jc in range(JC // 128):
                        nc.tensor.matmul(
                            acc[:],
                            lhsT=pwT_sb[:, jc, ot * 128 : (ot + 1) * 128],
                            rhs=xT_sb[:, jc, lh * 512 : (lh + 1) * 512],
                            start=(jc == 0),
                            stop=(jc == JC // 128 - 1),
                        )
                    nc.vector.tensor_copy(ob[:, lh * 512 : (lh + 1) * 512], acc[:])
                nc.sync.dma_start(out=outp[ot * 128 : (ot + 1) * 128, :], in_=ob[:])

    nc.compile()
    return nc


_NC_CACHE = None


def _get_nc():
    global _NC_CACHE
    if _NC_CACHE is None:
        _NC_CACHE = build_kernel()
    return _NC_CACHE


def _prep_inputs(q, kv, attn_pos, q_w, kv_w, proj_w):
    bf = ml_dtypes.bfloat16
    qws = (q_w.astype(np.float64) * (D ** -0.5)).astype(np.float32)
    in_maps = []
    for c in range(N_CORES):
        b, g = c // 2, c % 2
        js = slice(g * JC, (g + 1) * JC)
        in_maps.append(
            {
                "qT": np.ascontiguousarray(q[b].T).astype(bf),
                "kvT": np.ascontiguousarray(kv[b].T).astype(bf),
                "qwT": np.ascontiguousarray(qws[js].T).astype(bf),
                "kwT": np.ascontiguousarray(kv_w[js].T).astype(bf),
                "vwT": np.ascontiguousarray(kv_w[C + g * JC : C + (g + 1) * JC].T).astype(bf),
                "posT": np.ascontiguousarray(
                    attn_pos[b, g * HPC : (g + 1) * HPC].transpose(0, 2, 1)
                ).astype(bf),
                "pwT": np.ascontiguousarray(proj_w[:, js].T).astype(bf),
            }
        )
    return in_maps


def kernel(q, kv, attn_pos, q_w, kv_w, proj_w, proj_b, _trace=False):
    q = np.asarray(q, dtype=np.float32)
    kv = np.asarray(kv, dtype=np.float32)
    attn_pos = np.asarray(attn_pos, dtype=np.float32)
    q_w = np.asarray(q_w, dtype=np.float32)
    kv_w = np.asarray(kv_w, dtype=np.float32)
    proj_w = np.asarray(proj_w, dtype=np.float32)
    proj_b = np.asarray(proj_b, dtype=np.float32)

    nc = _get_nc()
    in_maps = _prep_inputs(q, kv, attn_pos, q_w, kv_w, proj_w)
    res = run_bass_kernel_spmd(nc, in_maps, core_ids=list(range(N_CORES)), trace=_trace)
    kernel.last_results = res

    out = np.empty((B, Lq, C), np.float32)
    for b in range(B):
        part = res.results[2 * b]["outp"] + res.results[2 * b + 1]["outp"]
        out[b] = part.T + proj_b[None, :]
    return out


if __name__ == "__main__":
    rng = np.random.default_rng(0)
    ins = {
        "q": rng.standard_normal((B, Lq, C), np.float32),
        "kv": rng.standard_normal((B, Lkv, C), np.float32),
        "attn_pos": rng.standard_normal((B, H, Lq, Lpos), np.float32),
        "q_w": rng.standard_normal((C, C), np.float32) * 0.02,
        "kv_w": rng.standard_normal((2 * C, C), np.float32) * 0.02,
        "proj_w": rng.standard_normal((C, C), np.float32) * 0.02,
        "proj_b": np.zeros((C,), np.float32),
    }
    out = kernel(**ins)
    print("out", out.shape, out.dtype, float(np.abs(out).mean()))


# revision 4
# speedup vs baseline: 1.1258x; 1.1258x over previous
"""Trainium2 Bass kernel for nn_CrossAttention (B=4, Lq=1024, Lkv=2048, C=1024, H=16).

Sharding (8 cores): core c -> batch b = c//2, head-group g = c%2 (8 of 16 heads).
Per-core TP over heads: q/k/v weights column-sharded, proj row-sharded; each core
computes a partial (C x Lq) projection output; host sums the pair and adds bias.

Device pipeline per core (all matmuls bf16 with fp32 PSUM accumulation):
  qhT  = (q_w_g * D^-0.5 @ q^T)            [512, 1024]   (j_local, l)
  kT   = (kw_g @ kv^T)                     [512, 2048]   (j_local, t)
  v    = (kv @ vw_g^T)                     [2048, 520]   (t, 8*65) with ones cols
  per head: S^T[t,l] = kT_h^T-slices x qhT_h   (K=64, 2-head row-packed)
            S^T += attn_pos^T (DVE add, t<1024)
            E = exp(S^T) (ACT, no max-subtraction: logits are O(5))
            O'aug^T[65,l] = v_aug^T x E  (ones row 64 = softmax denom Z)
            x^T_h = O'^T[0:64] * (1/Z)   (partition-broadcast recip)
  outp[o,l] = pw_g^T x x^T  (partial, summed across the core pair on host)
"""

import sys
import os

for _p in ("/opt/trn_rl_repo",):
    if _p not in sys.path and os.path.isdir(_p):
        sys.path.append(_p)

import numpy as np
import ml_dtypes

import concourse.bass as bass
import concourse.bacc as bacc
import concourse.mybir as mybir
from concourse.tile import TileContext
from concourse.bass_utils import run_bass_kernel_spmd

BF16 = mybir.dt.bfloat16
F32 = mybir.dt.float32
AF = mybir.ActivationFunctionType
ALU = mybir.AluOpType

B, Lq, Lkv, C, H, D, Lpos = 4, 1024, 2048, 1024, 16, 64, 1024
HPC = 8            # heads per core
JC = HPC * D       # 512: local head-dim width
N_CORES = 8
NT = Lkv // 128    # 16 t-tiles
NPOS = Lpos // 128  # 8 t-tiles carrying attn_pos


def build_kernel():
    nc = bacc.Bacc(trn_type="TRN2")

    qT = nc.declare_dram_parameter("qT", [C, Lq], BF16, isOutput=False)
    kvT = nc.declare_dram_parameter("kvT", [C, Lkv], BF16, isOutput=False)
    qwT = nc.declare_dram_parameter("qwT", [C, JC], BF16, isOutput=False)
    kwT = nc.declare_dram_parameter("kwT", [C, JC], BF16, isOutput=False)
    vwT = nc.declare_dram_parameter("vwT", [C, JC], BF16, isOutput=False)
    posT = nc.declare_dram_parameter("posT", [HPC, Lpos, Lq], BF16, isOutput=False)
    pwT = nc.declare_dram_parameter("pwT", [JC, C], BF16, isOutput=False)
    outp = nc.declare_dram_parameter("outp", [C, Lq], F32, isOutput=True)

    with TileContext(nc) as tc:
        with (
            tc.tile_pool(name="persist", bufs=1) as persist,
            tc.tile_pool(name="stage", bufs=1) as stage,
            tc.tile_pool(name="pos", bufs=4) as pospool,
            tc.tile_pool(name="exps", bufs=4) as expool,
            tc.tile_pool(name="small", bufs=4) as small,
            tc.tile_pool(name="osb", bufs=3) as osb,
            tc.tile_pool(name="ps", bufs=2, space="PSUM") as ps,
            tc.tile_pool(name="pso", bufs=2, space="PSUM") as pso,
        ):
            # ---- stage inputs ----
            qT_sb = stage.tile([128, C // 128, Lq], BF16)       # 2 MB
            nc.sync.dma_start(out=qT_sb[:], in_=qT.rearrange("(cc p) l -> p cc l", p=128))
            kvT_sb = stage.tile([128, C // 128, Lkv], BF16)     # 4 MB
            nc.sync.dma_start(out=kvT_sb[:], in_=kvT.rearrange("(cc p) t -> p cc t", p=128))
            qwT_sb = stage.tile([128, C // 128, JC], BF16)      # 1 MB
            nc.sync.dma_start(out=qwT_sb[:], in_=qwT.rearrange("(cc p) j -> p cc j", p=128))
            kwT_sb = stage.tile([128, C // 128, JC], BF16)
            nc.sync.dma_start(out=kwT_sb[:], in_=kwT.rearrange("(cc p) j -> p cc j", p=128))
            vwT_sb = stage.tile([128, C // 128, JC], BF16)
            nc.sync.dma_start(out=vwT_sb[:], in_=vwT.rearrange("(cc p) j -> p cc j", p=128))
            pwT_sb = stage.tile([128, JC // 128, C], BF16)      # 1 MB
            nc.sync.dma_start(out=pwT_sb[:], in_=pwT.rearrange("(jc p) o -> p jc o", p=128))

            # ---- persistent intermediates ----
            qhT_sb = persist.tile([128, JC // 128, Lq], BF16)   # (j%128, j//128, l)
            kT_sb = persist.tile([128, JC // 128, Lkv], BF16)   # (j%128, j//128, t)
            v_sb = persist.tile([128, NT, HPC * 65], BF16)      # (t%128, t//128, h*65+d; col 64 = ones)
            xT_sb = persist.tile([128, JC // 128, Lq], BF16)    # (j%128, j//128, l)

            # ones columns of v_aug (softmax denominator accumulators)
            for h in range(HPC):
                nc.gpsimd.memset(v_sb[:, :, h * 65 + 64 : h * 65 + 65], 1.0)

            NC = C // 128  # 8 contraction chunks

            def qh_group(p, lh):
                # qhT[j, l] for head-pair p, l-half lh
                acc = ps.tile([128, 512], F32, tag="s", name=f"qh_{p}_{lh}")
                for cc in range(NC):
                    nc.tensor.matmul(
                        acc[:],
                        lhsT=qwT_sb[:, cc, p * 128 : (p + 1) * 128],
                        rhs=qT_sb[:, cc, lh * 512 : (lh + 1) * 512],
                        start=(cc == 0),
                        stop=(cc == NC - 1),
                    )
                nc.vector.tensor_copy(qhT_sb[:, p, lh * 512 : (lh + 1) * 512], acc[:])

            def kt_group(p, tch):
                # kT[j, t] for head-pair p, 512-wide t-chunk tch
                acc = ps.tile([128, 512], F32, tag="s", name=f"kt_{p}_{tch}")
                for cc in range(NC):
                    nc.tensor.matmul(
                        acc[:],
                        lhsT=kwT_sb[:, cc, p * 128 : (p + 1) * 128],
                        rhs=kvT_sb[:, cc, tch * 512 : (tch + 1) * 512],
                        start=(cc == 0),
                        stop=(cc == NC - 1),
                    )
                nc.vector.tensor_copy(kT_sb[:, p, tch * 512 : (tch + 1) * 512], acc[:])

            def v_group(tt):
                # v[t, j] for all heads, t-tile tt
                acc = ps.tile([128, 512], F32, tag="s", name=f"v_{tt}")
                for cc in range(NC):
                    nc.tensor.matmul(
                        acc[:],
                        lhsT=kvT_sb[:, cc, tt * 128 : (tt + 1) * 128],
                        rhs=vwT_sb[:, cc, :],
                        start=(cc == 0),
                        stop=(cc == NC - 1),
                    )
                nc.vector.tensor_copy(
                    v_sb[:, tt, :].rearrange("p (h c) -> p h c", c=65)[:, :, 0:64],
                    acc[:].rearrange("p (h c) -> p h c", c=64),
                )

            # ---- upfront: v for all heads, q/k for pair 0 ----
            for tt in range(NT):
                v_group(tt)
            for lh in range(Lq // 512):
                qh_group(0, lh)
            for tch in range(Lkv // 512):
                kt_group(0, tch)

            # ---- attention, one head-pair at a time; next pair's q/k
            # projections are interleaved as PE filler so the tensor engine
            # stays dense (HAM stays at full clock) ----
            for p in range(HPC // 2):
                filler = []
                if p + 1 < HPC // 2:
                    filler = [(qh_group, (p + 1, lh)) for lh in range(2)] + [
                        (kt_group, (p + 1, tch)) for tch in range(4)
                    ]
                po = [
                    pso.tile([65, Lq], F32, tag="o", name=f"po{p}_{i}")
                    for i in range(2)
                ]
                for tt in range(NT):
                    s_tiles = []
                    for sub in range(2):  # head 2p+sub; row-packed K=64 matmuls
                        st = ps.tile([128, Lq], F32, tag="s", name=f"s{p}_{tt}_{sub}")
                        s_tiles.append(st)
                        lo, hi = sub * 64, sub * 64 + 64
                        for lh in range(Lq // 512):
                            nc.tensor.matmul(
                                st[:, lh * 512 : (lh + 1) * 512],
                                lhsT=kT_sb[lo:hi, p, tt * 128 : (tt + 1) * 128],
                                rhs=qhT_sb[lo:hi, p, lh * 512 : (lh + 1) * 512],
                                start=True,
                                stop=True,
                            )
                    for sub in range(2):
                        h = 2 * p + sub
                        st = s_tiles[sub]
                        if tt < NPOS:
                            pt = pospool.tile([128, Lq], BF16)
                            nc.sync.dma_start(
                                out=pt[:], in_=posT[h, tt * 128 : (tt + 1) * 128, :]
                            )
                            nc.vector.tensor_tensor(st[:], st[:], pt[:], op=ALU.add)
                        et = expool.tile([128, Lq], BF16)
                        nc.scalar.activation(et[:], st[:], AF.Exp)
                        for lh in range(Lq // 512):
                            nc.tensor.matmul(
                                po[sub][:, lh * 512 : (lh + 1) * 512],
                                lhsT=v_sb[:, tt, h * 65 : h * 65 + 65],
                                rhs=et[:, lh * 512 : (lh + 1) * 512],
                                start=(tt == 0),
                                stop=(tt == NT - 1),
                            )
                    if filler and tt % 3 == 2:
                        fn, args = filler.pop(0)
                        fn(*args)
                for fn, args in filler:
                    fn(*args)
                # normalize: x^T_h = O'[0:64] * 1/Z
                for sub in range(2):
                    z = small.tile([1, Lq], F32, tag="z")
                    nc.vector.tensor_copy(z[:], po[sub][64:65, :])
                    r = small.tile([1, Lq], F32, tag="r")
                    nc.vector.reciprocal_approx_fast(r[:], z[:])
                    r64 = small.tile([64, Lq], F32, tag="r64")
                    nc.gpsimd.partition_broadcast(r64[:], r[:])
                    nc.vector.tensor_tensor(
                        xT_sb[sub * 64 : sub * 64 + 64, p, :],
                        po[sub][0:64, :],
                        r64[:],
                        op=ALU.mult,
                    )

            # ---- output projection (partial): outp[o, l] ----
            for ot in range(C // 128):
                ob = osb.tile([128, Lq], F32)
                for lh in range(Lq // 512):
                    acc = ps.tile([128, 512], F32, tag="s")
                    for jc in range(JC // 128):
                        nc.tensor.matmul(
                            acc[:],
                            lhsT=pwT_sb[:, jc, ot * 128 : (ot + 1) * 128],
                            rhs=xT_sb[:, jc, lh * 512 : (lh + 1) * 512],
                            start=(jc == 0),
                            stop=(jc == JC // 128 - 1),
                        )
                    nc.vector.tensor_copy(ob[:, lh * 512 : (lh + 1) * 512], acc[:])
                nc.sync.dma_start(out=outp[ot * 128 : (ot + 1) * 128, :], in_=ob[:])

    nc.compile()
    return nc


_NC_CACHE = None


def _get_nc():
    global _NC_CACHE
    if _NC_CACHE is None:
        _NC_CACHE = build_kernel()
    return _NC_CACHE


def _prep_inputs(q, kv, attn_pos, q_w, kv_w, proj_w):
    bf = ml_dtypes.bfloat16
    qws = (q_w.astype(np.float64) * (D ** -0.5)).astype(np.float32)
    in_maps = []
    for c in range(N_CORES):
        b, g = c // 2, c % 2
        js = slice(g * JC, (g + 1) * JC)
        in_maps.append(
            {
                "qT": np.ascontiguousarray(q[b].T).astype(bf),
                "kvT": np.ascontiguousarray(kv[b].T).astype(bf),
                "qwT": np.ascontiguousarray(qws[js].T).astype(bf),
                "kwT": np.ascontiguousarray(kv_w[js].T).astype(bf),
                "vwT": np.ascontiguousarray(kv_w[C + g * JC : C + (g + 1) * JC].T).astype(bf),
                "posT": np.ascontiguousarray(
                    attn_pos[b, g * HPC : (g + 1) * HPC].transpose(0, 2, 1)
                ).astype(bf),
                "pwT": np.ascontiguousarray(proj_w[:, js].T).astype(bf),
            }
        )
    return in_maps


def kernel(q, kv, attn_pos, q_w, kv_w, proj_w, proj_b, _trace=False):
    q = np.asarray(q, dtype=np.float32)
    kv = np.asarray(kv, dtype=np.float32)
    attn_pos = np.asarray(attn_pos, dtype=np.float32)
    q_w = np.asarray(q_w, dtype=np.float32)
    kv_w = np.asarray(kv_w, dtype=np.float32)
    proj_w = np.asarray(proj_w, dtype=np.float32)
    proj_b = np.asarray(proj_b, dtype=np.float32)

    nc = _get_nc()
    in_maps = _prep_inputs(q, kv, attn_pos, q_w, kv_w, proj_w)
    res = run_bass_kernel_spmd(nc, in_maps, core_ids=list(range(N_CORES)), trace=_trace)
    kernel.last_results = res

    out = np.empty((B, Lq, C), np.float32)
    for b in range(B):
        part = res.results[2 * b]["outp"] + res.results[2 * b + 1]["outp"]
        out[b] = part.T + proj_b[None, :]
    return out


if __name__ == "__main__":
    rng = np.random.default_rng(0)
    ins = {
        "q": rng.standard_normal((B, Lq, C), np.float32),
        "kv": rng.standard_normal((B, Lkv, C), np.float32),
        "attn_pos": rng.standard_normal((B, H, Lq, Lpos), np.float32),
        "q_w": rng.standard_normal((C, C), np.float32) * 0.02,
        "kv_w": rng.standard_normal((2 * C, C), np.float32) * 0.02,
        "proj_w": rng.standard_normal((C, C), np.float32) * 0.02,
        "proj_b": np.zeros((C,), np.float32),
    }
    out = kernel(**ins)
    print("out", out.shape, out.dtype, float(np.abs(out).mean()))


# revision 9
# speedup vs baseline: 1.1607x; 1.0310x over previous
"""Trainium2 Bass kernel for nn_CrossAttention (B=4, Lq=1024, Lkv=2048, C=1024, H=16).

Sharding (8 cores): core c -> batch b = c//2, head-group g = c%2 (8 of 16 heads).
Per-core TP over heads: q/k/v weights column-sharded, proj row-sharded; each core
computes a partial (C x Lq) projection output; host sums the pair and adds bias.

Device pipeline per core (all matmuls bf16 with fp32 PSUM accumulation):
  qhT  = (q_w_g * D^-0.5 @ q^T)            [512, 1024]   (j_local, l)
  kT   = (kw_g @ kv^T)                     [512, 2048]   (j_local, t)
  v    = (kv @ vw_g^T)                     [2048, 520]   (t, 8*65) with ones cols
  per head: S^T[t,l] = kT_h^T-slices x qhT_h   (K=64, 2-head row-packed)
            S^T += attn_pos^T (DVE add, t<1024)
            E = exp(S^T) (ACT, no max-subtraction: logits are O(5))
            O'aug^T[65,l] = v_aug^T x E  (ones row 64 = softmax denom Z)
            x^T_h = O'^T[0:64] * (1/Z)   (partition-broadcast recip)
  outp[o,l] = pw_g^T x x^T  (partial, summed across the core pair on host)
"""

import sys
import os

for _p in ("/opt/trn_rl_repo",):
    if _p not in sys.path and os.path.isdir(_p):
        sys.path.append(_p)

import numpy as np
import ml_dtypes

import concourse.bass as bass
import concourse.bacc as bacc
import concourse.mybir as mybir
from concourse.tile import TileContext
from concourse.bass_utils import run_bass_kernel_spmd

BF16 = mybir.dt.bfloat16
F32 = mybir.dt.float32
AF = mybir.ActivationFunctionType
ALU = mybir.AluOpType

B, Lq, Lkv, C, H, D, Lpos = 4, 1024, 2048, 1024, 16, 64, 1024
HPC = 8            # heads per core
JC = HPC * D       # 512: local head-dim width
N_CORES = 8
NT = Lkv // 128    # 16 t-tiles
NPOS = Lpos // 128  # 8 t-tiles carrying attn_pos


def build_kernel():
    nc = bacc.Bacc(trn_type="TRN2")

    qT = nc.declare_dram_parameter("qT", [C, Lq], BF16, isOutput=False)
    kvT = nc.declare_dram_parameter("kvT", [C, Lkv], BF16, isOutput=False)
    qwT = nc.declare_dram_parameter("qwT", [C, JC], BF16, isOutput=False)
    kwT = nc.declare_dram_parameter("kwT", [C, JC], BF16, isOutput=False)
    vwT = nc.declare_dram_parameter("vwT", [C, JC], BF16, isOutput=False)
    posT = nc.declare_dram_parameter("posT", [HPC, Lpos, Lq], BF16, isOutput=False)
    pwT = nc.declare_dram_parameter("pwT", [JC, C], BF16, isOutput=False)
    outp = nc.declare_dram_parameter("outp", [C, Lq], F32, isOutput=True)

    from contextlib import ExitStack

    with TileContext(nc) as tc, ExitStack() as ctx:
        persist = ctx.enter_context(tc.tile_pool(name="persist", bufs=1))
        stageW = ctx.enter_context(tc.tile_pool(name="stageW", bufs=1))
        ps = ctx.enter_context(tc.tile_pool(name="ps", bufs=2, space="PSUM"))
        pso = ctx.enter_context(tc.tile_pool(name="pso", bufs=2, space="PSUM"))
        stageQ_cm = tc.tile_pool(name="stageQ", bufs=1)
        stageQ = stageQ_cm.__enter__()
        if True:
            # ---- stage inputs ----
            # long-lived staging (needed through the attention phase)
            kvT_sb = stageW.tile([128, C // 128, Lkv], BF16)    # 4 MB
            nc.sync.dma_start(out=kvT_sb[:], in_=kvT.rearrange("(cc p) t -> p cc t", p=128))
            kwT_sb = stageW.tile([128, C // 128, JC], BF16)     # 1 MB
            nc.sync.dma_start(out=kwT_sb[:], in_=kwT.rearrange("(cc p) j -> p cc j", p=128))
            pwT_sb = stageW.tile([128, JC // 128, C], BF16)     # 1 MB
            nc.sync.dma_start(out=pwT_sb[:], in_=pwT.rearrange("(jc p) o -> p jc o", p=128))
            # short-lived staging (freed after the q/v projections)
            qT_sb = stageQ.tile([128, C // 128, Lq], BF16)      # 2 MB
            nc.sync.dma_start(out=qT_sb[:], in_=qT.rearrange("(cc p) l -> p cc l", p=128))
            qwT_sb = stageQ.tile([128, C // 128, JC], BF16)     # 1 MB
            nc.sync.dma_start(out=qwT_sb[:], in_=qwT.rearrange("(cc p) j -> p cc j", p=128))
            vwT_sb = stageQ.tile([128, C // 128, JC], BF16)
            nc.sync.dma_start(out=vwT_sb[:], in_=vwT.rearrange("(cc p) j -> p cc j", p=128))

            # ---- persistent intermediates ----
            qhT_sb = persist.tile([128, JC // 128, Lq], BF16)   # (j%128, j//128, l)
            kT_sb = persist.tile([128, JC // 128, Lkv], BF16)   # (j%128, j//128, t)
            v_sb = persist.tile([128, NT, HPC * 65], BF16)      # (t%128, t//128, h*65+d; col 64 = ones)
            xT_sb = persist.tile([128, JC // 128, Lq], BF16)    # (j%128, j//128, l)

            # ones columns of v_aug (softmax denominator accumulators)
            for h in range(HPC):
                nc.gpsimd.memset(v_sb[:, :, h * 65 + 64 : h * 65 + 65], 1.0)

            NC = C // 128  # 8 contraction chunks

            def qh_group(p, lh):
                # qhT[j, l] for head-pair p, l-half lh
                acc = ps.tile([128, 512], F32, tag="s", name=f"qh_{p}_{lh}")
                for cc in range(NC):
                    nc.tensor.matmul(
                        acc[:],
                        lhsT=qwT_sb[:, cc, p * 128 : (p + 1) * 128],
                        rhs=qT_sb[:, cc, lh * 512 : (lh + 1) * 512],
                        start=(cc == 0),
                        stop=(cc == NC - 1),
                    )
                nc.vector.tensor_copy(qhT_sb[:, p, lh * 512 : (lh + 1) * 512], acc[:])

            def kt_group(p, tch):
                # kT[j, t] for head-pair p, 512-wide t-chunk tch
                acc = ps.tile([128, 512], F32, tag="s", name=f"kt_{p}_{tch}")
                for cc in range(NC):
                    nc.tensor.matmul(
                        acc[:],
                        lhsT=kwT_sb[:, cc, p * 128 : (p + 1) * 128],
                        rhs=kvT_sb[:, cc, tch * 512 : (tch + 1) * 512],
                        start=(cc == 0),
                        stop=(cc == NC - 1),
                    )
                nc.vector.tensor_copy(kT_sb[:, p, tch * 512 : (tch + 1) * 512], acc[:])

            def v_group(tt):
                # v[t, j] for all heads, t-tile tt
                acc = ps.tile([128, 512], F32, tag="s", name=f"v_{tt}")
                for cc in range(NC):
                    nc.tensor.matmul(
                        acc[:],
                        lhsT=kvT_sb[:, cc, tt * 128 : (tt + 1) * 128],
                        rhs=vwT_sb[:, cc, :],
                        start=(cc == 0),
                        stop=(cc == NC - 1),
                    )
                nc.vector.tensor_copy(
                    v_sb[:, tt, :].rearrange("p (h c) -> p h c", c=65)[:, :, 0:64],
                    acc[:].rearrange("p (h c) -> p h c", c=64),
                )

            # ---- upfront: v + q projections for all heads, k for pair 0 ----
            for tt in range(NT):
                v_group(tt)
            for p in range(HPC // 2):
                for lh in range(Lq // 512):
                    qh_group(p, lh)
            for tch in range(Lkv // 512):
                kt_group(0, tch)

            # q-side staging no longer needed; free its SBUF for the deep
            # exp(S) buffers below
            stageQ_cm.__exit__(None, None, None)
            pospool = ctx.enter_context(tc.tile_pool(name="pos", bufs=3))
            expsP = ctx.enter_context(tc.tile_pool(name="expsP", bufs=2))
            small = ctx.enter_context(tc.tile_pool(name="small", bufs=2))
            osb = ctx.enter_context(tc.tile_pool(name="osb", bufs=2))

            # ---- attention, one head at a time ----
            # A full head of exp(S) is buffered in SBUF (16 t-tiles deep,
            # double-buffered across heads) so the O' matmuls run as a dense
            # back-to-back burst while the next head's S/exp stream runs.
            # Next pair's k projections interleave as extra PE filler.
            for h in range(HPC):
                p, sub = h // 2, h % 2
                lo, hi = sub * 64, sub * 64 + 64
                filler = []
                if sub == 0 and p + 1 < HPC // 2:
                    filler = [(kt_group, (p + 1, tch)) for tch in range(4)]
                es = expsP.tile([128, NT, Lq], BF16, tag="es", name=f"exps{h}")
                po = pso.tile([65, Lq], F32, tag="o", name=f"po{h}")
                for tt in range(NT):
                    st = ps.tile([128, Lq], F32, tag="s", name=f"s{h}_{tt}")
                    for lh in range(Lq // 512):
                        nc.tensor.matmul(
                            st[:, lh * 512 : (lh + 1) * 512],
                            lhsT=kT_sb[lo:hi, p, tt * 128 : (tt + 1) * 128],
                            rhs=qhT_sb[lo:hi, p, lh * 512 : (lh + 1) * 512],
                            start=True,
                            stop=True,
                        )
                    if tt < NPOS:
                        pt = pospool.tile([128, Lq], BF16)
                        nc.sync.dma_start(
                            out=pt[:], in_=posT[h, tt * 128 : (tt + 1) * 128, :]
                        )
                        nc.vector.tensor_tensor(st[:], st[:], pt[:], op=ALU.add)
                    nc.scalar.activation(es[:, tt, :], st[:], AF.Exp)
                    if filler and tt % 4 == 3:
                        fn, args = filler.pop(0)
                        fn(*args)
                for tt in range(NT):
                    for lh in range(Lq // 512):
                        nc.tensor.matmul(
                            po[:, lh * 512 : (lh + 1) * 512],
                            lhsT=v_sb[:, tt, h * 65 : h * 65 + 65],
                            rhs=es[:, tt, lh * 512 : (lh + 1) * 512],
                            start=(tt == 0),
                            stop=(tt == NT - 1),
                        )
                for fn, args in filler:
                    fn(*args)
                # normalize: x^T_h = O'[0:64] * 1/Z
                z = small.tile([1, Lq], F32, tag="z")
                nc.vector.tensor_copy(z[:], po[64:65, :])
                r = small.tile([1, Lq], F32, tag="r")
                nc.vector.reciprocal_approx_fast(r[:], z[:])
                r64 = small.tile([64, Lq], F32, tag="r64")
                nc.gpsimd.partition_broadcast(r64[:], r[:])
                nc.vector.tensor_tensor(
                    xT_sb[lo:hi, p, :], po[0:64, :], r64[:], op=ALU.mult
                )

            # ---- output projection (partial): outp[o, l] ----
            for ot in range(C // 128):
                ob = osb.tile([128, Lq], F32)
                for lh in range(Lq // 512):
                    acc = ps.tile([128, 512], F32, tag="s")
                    for jc in range(JC // 128):
                        nc.tensor.matmul(
                            acc[:],
                            lhsT=pwT_sb[:, jc, ot * 128 : (ot + 1) * 128],
                            rhs=xT_sb[:, jc, lh * 512 : (lh + 1) * 512],
                            start=(jc == 0),
                            stop=(jc == JC // 128 - 1),
                        )
                    nc.vector.tensor_copy(ob[:, lh * 512 : (lh + 1) * 512], acc[:])
                nc.sync.dma_start(out=outp[ot * 128 : (ot + 1) * 128, :], in_=ob[:])

    nc.compile()
    return nc


_NC_CACHE = None


def _get_nc():
    global _NC_CACHE
    if _NC_CACHE is None:
        _NC_CACHE = build_kernel()
    return _NC_CACHE


def _prep_inputs(q, kv, attn_pos, q_w, kv_w, proj_w):
    bf = ml_dtypes.bfloat16
    qws = (q_w.astype(np.float64) * (D ** -0.5)).astype(np.float32)
    in_maps = []
    for c in range(N_CORES):
        b, g = c // 2, c % 2
        js = slice(g * JC, (g + 1) * JC)
        in_maps.append(
            {
                "qT": np.ascontiguousarray(q[b].T).astype(bf),
                "kvT": np.ascontiguousarray(kv[b].T).astype(bf),
                "qwT": np.ascontiguousarray(qws[js].T).astype(bf),
                "kwT": np.ascontiguousarray(kv_w[js].T).astype(bf),
                "vwT": np.ascontiguousarray(kv_w[C + g * JC : C + (g + 1) * JC].T).astype(bf),
                "posT": np.ascontiguousarray(
                    attn_pos[b, g * HPC : (g + 1) * HPC].transpose(0, 2, 1)
                ).astype(bf),
                "pwT": np.ascontiguousarray(proj_w[:, js].T).astype(bf),
            }
        )
    return in_maps


def kernel(q, kv, attn_pos, q_w, kv_w, proj_w, proj_b, _trace=False):
    q = np.asarray(q, dtype=np.float32)
    kv = np.asarray(kv, dtype=np.float32)
    attn_pos = np.asarray(attn_pos, dtype=np.float32)
    q_w = np.asarray(q_w, dtype=np.float32)
    kv_w = np.asarray(kv_w, dtype=np.float32)
    proj_w = np.asarray(proj_w, dtype=np.float32)
    proj_b = np.asarray(proj_b, dtype=np.float32)

    nc = _get_nc()
    in_maps = _prep_inputs(q, kv, attn_pos, q_w, kv_w, proj_w)
    res = run_bass_kernel_spmd(nc, in_maps, core_ids=list(range(N_CORES)), trace=_trace)
    kernel.last_results = res

    out = np.empty((B, Lq, C), np.float32)
    for b in range(B):
        part = res.results[2 * b]["outp"] + res.results[2 * b + 1]["outp"]
        out[b] = part.T + proj_b[None, :]
    return out


if __name__ == "__main__":
    rng = np.random.default_rng(0)
    ins = {
        "q": rng.standard_normal((B, Lq, C), np.float32),
        "kv": rng.standard_normal((B, Lkv, C), np.float32),
        "attn_pos": rng.standard_normal((B, H, Lq, Lpos), np.float32),
        "q_w": rng.standard_normal((C, C), np.float32) * 0.02,
        "kv_w": rng.standard_normal((2 * C, C), np.float32) * 0.02,
        "proj_w": rng.standard_normal((C, C), np.float32) * 0.02,
        "proj_b": np.zeros((C,), np.float32),
    }
    out = kernel(**ins)
    print("out", out.shape, out.dtype, float(np.abs(out).mean()))


# revision 11
# speedup vs baseline: 1.3097x; 1.1284x over previous
"""Trainium2 Bass kernel for nn_CrossAttention (B=4, Lq=1024, Lkv=2048, C=1024, H=16).

Sharding (8 cores): core c -> batch b = c//2, head-group g = c%2 (8 of 16 heads).
Per-core TP over heads: q/k/v weights column-sharded, proj row-sharded; each core
computes a partial (C x Lq) projection output; host sums the pair and adds bias.

Device pipeline per core (all matmuls bf16 with fp32 PSUM accumulation):
  qhT  = (q_w_g * D^-0.5 @ q^T)            [512, 1024]   (j_local, l)
  kT   = (kw_g @ kv^T)                     [512, 2048]   (j_local, t)
  v    = (kv @ vw_g^T)                     [2048, 520]   (t, 8*65) with ones cols
  per head: S^T[t,l] = kT_h^T-slices x qhT_h   (K=64, 2-head row-packed)
            S^T += attn_pos^T (DVE add, t<1024)
            E = exp(S^T) (ACT, no max-subtraction: logits are O(5))
            O'aug^T[65,l] = v_aug^T x E  (ones row 64 = softmax denom Z)
            x^T_h = O'^T[0:64] * (1/Z)   (partition-broadcast recip)
  outp[o,l] = pw_g^T x x^T  (partial, summed across the core pair on host)
"""

import sys
import os

for _p in ("/opt/trn_rl_repo",):
    if _p not in sys.path and os.path.isdir(_p):
        sys.path.append(_p)

import numpy as np
import ml_dtypes

import concourse.bass as bass
import concourse.bacc as bacc
import concourse.mybir as mybir
from concourse.tile import TileContext
from concourse.bass_utils import run_bass_kernel_spmd

BF16 = mybir.dt.bfloat16
F32 = mybir.dt.float32
AF = mybir.ActivationFunctionType
ALU = mybir.AluOpType

B, Lq, Lkv, C, H, D, Lpos = 4, 1024, 2048, 1024, 16, 64, 1024
HPC = 8            # heads per core
JC = HPC * D       # 512: local head-dim width
N_CORES = 8
NT = Lkv // 128    # 16 t-tiles
NPOS = Lpos // 128  # 8 t-tiles carrying attn_pos


def build_kernel():
    nc = bacc.Bacc(trn_type="TRN2")

    qT = nc.declare_dram_parameter("qT", [C, Lq], BF16, isOutput=False)
    kvT = nc.declare_dram_parameter("kvT", [C, Lkv], BF16, isOutput=False)
    qwT = nc.declare_dram_parameter("qwT", [C, JC], BF16, isOutput=False)
    kwT = nc.declare_dram_parameter("kwT", [C, JC], BF16, isOutput=False)
    vwT = nc.declare_dram_parameter("vwT", [C, JC], BF16, isOutput=False)
    posT = nc.declare_dram_parameter("posT", [HPC, Lpos, Lq], BF16, isOutput=False)
    pwT = nc.declare_dram_parameter("pwT", [JC, C], BF16, isOutput=False)
    outp = nc.declare_dram_parameter("outp", [C, Lq], F32, isOutput=True)

    from contextlib import ExitStack

    with TileContext(nc) as tc, ExitStack() as ctx:
        persist = ctx.enter_context(tc.tile_pool(name="persist", bufs=1))
        stageW = ctx.enter_context(tc.tile_pool(name="stageW", bufs=1))
        ps = ctx.enter_context(tc.tile_pool(name="ps", bufs=2, space="PSUM"))
        pso = ctx.enter_context(tc.tile_pool(name="pso", bufs=1, space="PSUM"))
        stageQ_cm = tc.tile_pool(name="stageQ", bufs=1)
        stageQ = stageQ_cm.__enter__()
        if True:
            # ---- stage inputs ----
            # long-lived staging (needed through the attention phase)
            kvT_sb = []
            for cc in range(C // 128):
                t = stageW.tile([128, Lkv], BF16, tag="kvT", name=f"kvT{cc}", bufs=C // 128)
                nc.sync.dma_start(out=t[:], in_=kvT[cc * 128 : (cc + 1) * 128, :])
                kvT_sb.append(t)
            kwT_sb = stageW.tile([128, C // 128, JC], BF16)     # 1 MB
            nc.sync.dma_start(out=kwT_sb[:], in_=kwT.rearrange("(cc p) j -> p cc j", p=128))
            pwT_sb = stageW.tile([128, JC // 128, C], BF16)     # 1 MB
            nc.sync.dma_start(out=pwT_sb[:], in_=pwT.rearrange("(jc p) o -> p jc o", p=128))
            # short-lived staging (freed after the q/v projections)
            qT_sb = []
            for cc in range(C // 128):
                t = stageQ.tile([128, Lq], BF16, tag="qT", name=f"qT{cc}", bufs=C // 128)
                nc.sync.dma_start(out=t[:], in_=qT[cc * 128 : (cc + 1) * 128, :])
                qT_sb.append(t)
            qwT_sb = stageQ.tile([128, C // 128, JC], BF16)     # 1 MB
            nc.sync.dma_start(out=qwT_sb[:], in_=qwT.rearrange("(cc p) j -> p cc j", p=128))
            vwT_sb = stageQ.tile([128, C // 128, JC], BF16)
            nc.sync.dma_start(out=vwT_sb[:], in_=vwT.rearrange("(cc p) j -> p cc j", p=128))

            # ---- persistent intermediates ----
            qhT_sb = persist.tile([128, JC // 128, Lq], BF16)   # (j%128, j//128, l)
            kT_sb = persist.tile([128, JC // 128, Lkv], BF16)   # (j%128, j//128, t)
            v_sb = persist.tile([128, NT, HPC * 65], BF16)      # (t%128, t//128, h*65+d; col 64 = ones)
            xT_sb = persist.tile([128, JC // 128, Lq], BF16)    # (j%128, j//128, l)

            # ones columns of v_aug (softmax denominator accumulators)
            for h in range(HPC):
                nc.gpsimd.memset(v_sb[:, :, h * 65 + 64 : h * 65 + 65], 1.0)

            NC = C // 128  # 8 contraction chunks

            def qh_group(p, lh):
                # qhT[j, l] for head-pair p, l-half lh
                acc = ps.tile([128, 512], F32, tag="pj", name=f"qh_{p}_{lh}")
                for cc in range(NC):
                    nc.tensor.matmul(
                        acc[:],
                        lhsT=qwT_sb[:, cc, p * 128 : (p + 1) * 128],
                        rhs=qT_sb[cc][:, lh * 512 : (lh + 1) * 512],
                        start=(cc == 0),
                        stop=(cc == NC - 1),
                    )
                nc.vector.tensor_copy(qhT_sb[:, p, lh * 512 : (lh + 1) * 512], acc[:])

            def kt_group(p, tch):
                # kT[j, t] for head-pair p, 512-wide t-chunk tch
                acc = ps.tile([128, 512], F32, tag="pj", name=f"kt_{p}_{tch}")
                for cc in range(NC):
                    nc.tensor.matmul(
                        acc[:],
                        lhsT=kwT_sb[:, cc, p * 128 : (p + 1) * 128],
                        rhs=kvT_sb[cc][:, tch * 512 : (tch + 1) * 512],
                        start=(cc == 0),
                        stop=(cc == NC - 1),
                    )
                nc.vector.tensor_copy(kT_sb[:, p, tch * 512 : (tch + 1) * 512], acc[:])

            def v_group(tt):
                # v[t, j] for all heads, t-tile tt
                acc = ps.tile([128, 512], F32, tag="pj", name=f"v_{tt}")
                for cc in range(NC):
                    nc.tensor.matmul(
                        acc[:],
                        lhsT=kvT_sb[cc][:, tt * 128 : (tt + 1) * 128],
                        rhs=vwT_sb[:, cc, :],
                        start=(cc == 0),
                        stop=(cc == NC - 1),
                    )
                nc.vector.tensor_copy(
                    v_sb[:, tt, :].rearrange("p (h c) -> p h c", c=65)[:, :, 0:64],
                    acc[:].rearrange("p (h c) -> p h c", c=64),
                )

            # ---- upfront: pair-0 projections, then v for all heads ----
            for tch in range(Lkv // 512):
                kt_group(0, tch)
            for lh in range(Lq // 512):
                qh_group(0, lh)
            for tt in range(NT):
                v_group(tt)
            for p in range(1, HPC // 2):
                for lh in range(2):
                    qh_group(p, lh)

            # q-side staging no longer needed; free its SBUF for the deep
            # exp(S) buffers below
            stageQ_cm.__exit__(None, None, None)
            pospool = ctx.enter_context(tc.tile_pool(name="pos", bufs=3))
            expsP = ctx.enter_context(tc.tile_pool(name="expsP", bufs=2))
            small = ctx.enter_context(tc.tile_pool(name="small", bufs=2))
            osb = ctx.enter_context(tc.tile_pool(name="osb", bufs=2))

            # ---- attention, one head at a time ----
            # A full head of exp(S) is buffered in SBUF (16 t-tiles deep,
            # double-buffered across heads) so the O' matmuls run as a dense
            # back-to-back burst while the next head's S/exp stream runs.
            # Next pair's k projections interleave as extra PE filler.
            for h in range(HPC):
                p, sub = h // 2, h % 2
                lo, hi = sub * 64, sub * 64 + 64
                filler = []
                if sub == 0 and p + 1 < HPC // 2:
                    filler = [(kt_group, (p + 1, tch)) for tch in range(4)]
                es = expsP.tile([128, NT, Lq], BF16, tag="es", name=f"exps{h}")
                po = pso.tile([65, Lq], F32, tag="o", name=f"po{h}")
                for tt in range(NT):
                    st = ps.tile([128, Lq], F32, tag="s", name=f"s{h}_{tt}")
                    for lh in range(Lq // 512):
                        nc.tensor.matmul(
                            st[:, lh * 512 : (lh + 1) * 512],
                            lhsT=kT_sb[lo:hi, p, tt * 128 : (tt + 1) * 128],
                            rhs=qhT_sb[lo:hi, p, lh * 512 : (lh + 1) * 512],
                            start=True,
                            stop=True,
                        )
                    if tt < NPOS:
                        pt = pospool.tile([128, Lq], BF16)
                        nc.sync.dma_start(
                            out=pt[:], in_=posT[h, tt * 128 : (tt + 1) * 128, :]
                        )
                        nc.vector.tensor_tensor(st[:], st[:], pt[:], op=ALU.add)
                    nc.scalar.activation(es[:, tt, :], st[:], AF.Exp)
                    if filler and tt % 3 == 2:
                        fn, args = filler.pop(0)
                        fn(*args)
                for tt in range(NT):
                    for lh in range(Lq // 512):
                        nc.tensor.matmul(
                            po[:, lh * 512 : (lh + 1) * 512],
                            lhsT=v_sb[:, tt, h * 65 : h * 65 + 65],
                            rhs=es[:, tt, lh * 512 : (lh + 1) * 512],
                            start=(tt == 0),
                            stop=(tt == NT - 1),
                        )
                for fn, args in filler:
                    fn(*args)
                # normalize: x^T_h = O'[0:64] * 1/Z
                z = small.tile([1, Lq], F32, tag="z")
                nc.vector.tensor_copy(z[:], po[64:65, :])
                r = small.tile([1, Lq], F32, tag="r")
                nc.vector.reciprocal_approx_fast(r[:], z[:])
                r64 = small.tile([64, Lq], F32, tag="r64")
                nc.gpsimd.partition_broadcast(r64[:], r[:])
                nc.vector.tensor_tensor(
                    xT_sb[lo:hi, p, :], po[0:64, :], r64[:], op=ALU.mult
                )

            # ---- output projection (partial): outp[o, l] ----
            for ot in range(C // 128):
                ob = osb.tile([128, Lq], F32)
                for lh in range(Lq // 512):
                    acc = ps.tile([128, 512], F32, tag="pj", name=f"o4_{ot}_{lh}")
                    for jc in range(JC // 128):
                        nc.tensor.matmul(
                            acc[:],
                            lhsT=pwT_sb[:, jc, ot * 128 : (ot + 1) * 128],
                            rhs=xT_sb[:, jc, lh * 512 : (lh + 1) * 512],
                            start=(jc == 0),
                            stop=(jc == JC // 128 - 1),
                        )
                    nc.vector.tensor_copy(ob[:, lh * 512 : (lh + 1) * 512], acc[:])
                nc.sync.dma_start(out=outp[ot * 128 : (ot + 1) * 128, :], in_=ob[:])

    nc.compile()
    return nc


_NC_CACHE = None


def _get_nc():
    global _NC_CACHE
    if _NC_CACHE is None:
        _NC_CACHE = build_kernel()
    return _NC_CACHE


def _prep_inputs(q, kv, attn_pos, q_w, kv_w, proj_w):
    bf = ml_dtypes.bfloat16
    qws = (q_w.astype(np.float64) * (D ** -0.5)).astype(np.float32)
    in_maps = []
    for c in range(N_CORES):
        b, g = c // 2, c % 2
        js = slice(g * JC, (g + 1) * JC)
        in_maps.append(
            {
                "qT": np.ascontiguousarray(q[b].T).astype(bf),
                "kvT": np.ascontiguousarray(kv[b].T).astype(bf),
                "qwT": np.ascontiguousarray(qws[js].T).astype(bf),
                "kwT": np.ascontiguousarray(kv_w[js].T).astype(bf),
                "vwT": np.ascontiguousarray(kv_w[C + g * JC : C + (g + 1) * JC].T).astype(bf),
                "posT": np.ascontiguousarray(
                    attn_pos[b, g * HPC : (g + 1) * HPC].transpose(0, 2, 1)
                ).astype(bf),
                "pwT": np.ascontiguousarray(proj_w[:, js].T).astype(bf),
            }
        )
    return in_maps


def kernel(q, kv, attn_pos, q_w, kv_w, proj_w, proj_b, _trace=False):
    q = np.asarray(q, dtype=np.float32)
    kv = np.asarray(kv, dtype=np.float32)
    attn_pos = np.asarray(attn_pos, dtype=np.float32)
    q_w = np.asarray(q_w, dtype=np.float32)
    kv_w = np.asarray(kv_w, dtype=np.float32)
    proj_w = np.asarray(proj_w, dtype=np.float32)
    proj_b = np.asarray(proj_b, dtype=np.float32)

    nc = _get_nc()
    in_maps = _prep_inputs(q, kv, attn_pos, q_w, kv_w, proj_w)
    res = run_bass_kernel_spmd(nc, in_maps, core_ids=list(range(N_CORES)), trace=_trace)
    kernel.last_results = res

    out = np.empty((B, Lq, C), np.float32)
    for b in range(B):
        part = res.results[2 * b]["outp"] + res.results[2 * b + 1]["outp"]
        out[b] = part.T + proj_b[None, :]
    return out


if __name__ == "__main__":
    rng = np.random.default_rng(0)
    ins = {
        "q": rng.standard_normal((B, Lq, C), np.float32),
        "kv": rng.standard_normal((B, Lkv, C), np.float32),
        "attn_pos": rng.standard_normal((B, H, Lq, Lpos), np.float32),
        "q_w": rng.standard_normal((C, C), np.float32) * 0.02,
        "kv_w": rng.standard_normal((2 * C, C), np.float32) * 0.02,
        "proj_w": rng.standard_normal((C, C), np.float32) * 0.02,
        "proj_b": np.zeros((C,), np.float32),
    }
    out = kernel(**ins)
    print("out", out.shape, out.dtype, float(np.abs(out).mean()))


# revision 13
# speedup vs baseline: 1.4477x; 1.1053x over previous
"""Trainium2 Bass kernel for nn_CrossAttention (B=4, Lq=1024, Lkv=2048, C=1024, H=16).

Sharding (8 cores): core c -> batch b = c//2, head-group g = c%2 (8 of 16 heads).
Per-core TP over heads: q/k/v weights column-sharded, proj row-sharded; each core
computes a partial (C x Lq) projection output; host sums the pair and adds bias.

Device pipeline per core (all matmuls bf16 with fp32 PSUM accumulation):
  qhT  = (q_w_g * D^-0.5 @ q^T)            [512, 1024]   (j_local, l)
  kT   = (kw_g @ kv^T)                     [512, 2048]   (j_local, t)
  v    = (kv @ vw_g^T)                     [2048, 520]   (t, 8*65) with ones cols
  per head: S^T[t,l] = kT_h^T-slices x qhT_h   (K=64, 2-head row-packed)
            S^T += attn_pos^T (DVE add, t<1024)
            E = exp(S^T) (ACT, no max-subtraction: logits are O(5))
            O'aug^T[65,l] = v_aug^T x E  (ones row 64 = softmax denom Z)
            x^T_h = O'^T[0:64] * (1/Z)   (partition-broadcast recip)
  outp[o,l] = pw_g^T x x^T  (partial, summed across the core pair on host)
"""

import sys
import os

for _p in ("/opt/trn_rl_repo",):
    if _p not in sys.path and os.path.isdir(_p):
        sys.path.append(_p)

import numpy as np
import ml_dtypes

import concourse.bass as bass
import concourse.bacc as bacc
import concourse.mybir as mybir
from concourse.tile import TileContext
from concourse.bass_utils import run_bass_kernel_spmd

BF16 = mybir.dt.bfloat16
F32 = mybir.dt.float32
AF = mybir.ActivationFunctionType
ALU = mybir.AluOpType

B, Lq, Lkv, C, H, D, Lpos = 4, 1024, 2048, 1024, 16, 64, 1024
HPC = 8            # heads per core
JC = HPC * D       # 512: local head-dim width
N_CORES = 8
NT = Lkv // 128    # 16 t-tiles
NPOS = Lpos // 128  # 8 t-tiles carrying attn_pos


def build_kernel():
    nc = bacc.Bacc(trn_type="TRN2")

    qT = nc.declare_dram_parameter("qT", [C, Lq], BF16, isOutput=False)
    kvT = nc.declare_dram_parameter("kvT", [C, Lkv], BF16, isOutput=False)
    qwT = nc.declare_dram_parameter("qwT", [C, JC], BF16, isOutput=False)
    kwT = nc.declare_dram_parameter("kwT", [C, JC], BF16, isOutput=False)
    vwT = nc.declare_dram_parameter("vwT", [C, JC], BF16, isOutput=False)
    posT = nc.declare_dram_parameter("posT", [HPC, Lpos, Lq], BF16, isOutput=False)
    pwT = nc.declare_dram_parameter("pwT", [JC, C], BF16, isOutput=False)
    outp = nc.declare_dram_parameter("outp", [C, Lq], F32, isOutput=True)

    from contextlib import ExitStack

    with TileContext(nc) as tc, ExitStack() as ctx:
        persist = ctx.enter_context(tc.tile_pool(name="persist", bufs=1))
        stageW = ctx.enter_context(tc.tile_pool(name="stageW", bufs=1))
        stageQ = ctx.enter_context(tc.tile_pool(name="stageQ", bufs=1))
        ps = ctx.enter_context(tc.tile_pool(name="ps", bufs=2, space="PSUM"))
        pso = ctx.enter_context(tc.tile_pool(name="pso", bufs=1, space="PSUM"))
        pospool = ctx.enter_context(tc.tile_pool(name="pos", bufs=3))
        expsP = ctx.enter_context(tc.tile_pool(name="expsP", bufs=2))
        small = ctx.enter_context(tc.tile_pool(name="small", bufs=1))
        osb = ctx.enter_context(tc.tile_pool(name="osb", bufs=2))
        if True:
            # ---- stage inputs (k-side first: first matmuls need kwT + kvT) ----
            kwT_sb = stageW.tile([128, C // 128, JC], BF16)
            nc.sync.dma_start(out=kwT_sb[:], in_=kwT.rearrange("(cc p) j -> p cc j", p=128))
            kvT_sb = []
            for cc in range(C // 128):
                t = stageW.tile([128, Lkv], BF16, tag="kvT", name=f"kvT{cc}", bufs=C // 128)
                nc.sync.dma_start(out=t[:], in_=kvT[cc * 128 : (cc + 1) * 128, :])
                kvT_sb.append(t)
            qwT_sb = stageQ.tile([128, C // 128, JC], BF16)
            nc.sync.dma_start(out=qwT_sb[:], in_=qwT.rearrange("(cc p) j -> p cc j", p=128))
            qT_sb = []
            for cc in range(C // 128):
                t = stageQ.tile([128, Lq], BF16, tag="qT", name=f"qT{cc}", bufs=C // 128)
                nc.sync.dma_start(out=t[:], in_=qT[cc * 128 : (cc + 1) * 128, :])
                qT_sb.append(t)
            vwT_sb = stageW.tile([128, C // 128, JC], BF16)
            nc.sync.dma_start(out=vwT_sb[:], in_=vwT.rearrange("(cc p) j -> p cc j", p=128))
            pwT_sb = stageW.tile([128, JC // 128, C], BF16)
            nc.sync.dma_start(out=pwT_sb[:], in_=pwT.rearrange("(jc p) o -> p jc o", p=128))

            # ---- persistent intermediates ----
            qhT_sb = persist.tile([128, JC // 128, Lq], BF16)   # (j%128, j//128, l)
            kT_sb = persist.tile([128, JC // 128, Lkv], BF16)   # (j%128, j//128, t)
            v_sb = persist.tile([128, NT, HPC * 65], BF16)      # (t%128, t//128, h*65+d; col 64 = ones)
            xT_sb = persist.tile([128, JC // 128, Lq], BF16)    # (j%128, j//128, l)

            # ones columns of v_aug (softmax denominator accumulators)
            for h in range(HPC):
                nc.gpsimd.memset(v_sb[:, :, h * 65 + 64 : h * 65 + 65], 1.0)

            NC = C // 128  # 8 contraction chunks

            def qh_group(p, lh):
                # qhT[j, l] for head-pair p, l-half lh
                acc = ps.tile([128, 512], F32, tag="pj", name=f"qh_{p}_{lh}")
                for cc in range(NC):
                    nc.tensor.matmul(
                        acc[:],
                        lhsT=qwT_sb[:, cc, p * 128 : (p + 1) * 128],
                        rhs=qT_sb[cc][:, lh * 512 : (lh + 1) * 512],
                        start=(cc == 0),
                        stop=(cc == NC - 1),
                    )
                nc.vector.tensor_copy(qhT_sb[:, p, lh * 512 : (lh + 1) * 512], acc[:])

            def kt_group(p, tch):
                # kT[j, t] for head-pair p, 512-wide t-chunk tch
                acc = ps.tile([128, 512], F32, tag="pj", name=f"kt_{p}_{tch}")
                for cc in range(NC):
                    nc.tensor.matmul(
                        acc[:],
                        lhsT=kwT_sb[:, cc, p * 128 : (p + 1) * 128],
                        rhs=kvT_sb[cc][:, tch * 512 : (tch + 1) * 512],
                        start=(cc == 0),
                        stop=(cc == NC - 1),
                    )
                nc.vector.tensor_copy(kT_sb[:, p, tch * 512 : (tch + 1) * 512], acc[:])

            def v_group(tt):
                # v[t, j] for all heads, t-tile tt
                acc = ps.tile([128, 512], F32, tag="pj", name=f"v_{tt}")
                for cc in range(NC):
                    nc.tensor.matmul(
                        acc[:],
                        lhsT=kvT_sb[cc][:, tt * 128 : (tt + 1) * 128],
                        rhs=vwT_sb[:, cc, :],
                        start=(cc == 0),
                        stop=(cc == NC - 1),
                    )
                nc.vector.tensor_copy(
                    v_sb[:, tt, :].rearrange("p (h c) -> p h c", c=65)[:, :, 0:64],
                    acc[:].rearrange("p (h c) -> p h c", c=64),
                )

            # ---- upfront: pair-0 q/k projections; everything else (v, other
            # pairs' q/k) streams into the attention loop as PE filler.
            # Tile schedules in program order, so overlap must be expressed in
            # emission order. ----
            for tch in range(Lkv // 512):
                kt_group(0, tch)
            for lh in range(Lq // 512):
                qh_group(0, lh)
            fillers = (
                [(v_group, (tt,)) for tt in range(NT)]
                + [(qh_group, (p, lh)) for p in (1, 2, 3) for lh in (0, 1)]
                + [(kt_group, (p, tch)) for p in (1, 2, 3) for tch in range(4)]
            )

            # ---- attention, one head at a time ----
            # exp(S) is buffered half-head deep (8 t-tiles, double-buffered)
            # so the O' matmuls run as dense back-to-back bursts.
            for h in range(HPC):
                p, sub = h // 2, h % 2
                lo, hi = sub * 64, sub * 64 + 64
                po = pso.tile([65, Lq], F32, tag="o", name=f"po{h}")
                for half in range(2):
                    es = expsP.tile([128, NT // 2, Lq], BF16, tag="es", name=f"es{h}_{half}")
                    for tq in range(NT // 2):
                        tt = half * (NT // 2) + tq
                        st = ps.tile([128, Lq], F32, tag="s", name=f"s{h}_{tt}")
                        for lh in range(Lq // 512):
                            nc.tensor.matmul(
                                st[:, lh * 512 : (lh + 1) * 512],
                                lhsT=kT_sb[lo:hi, p, tt * 128 : (tt + 1) * 128],
                                rhs=qhT_sb[lo:hi, p, lh * 512 : (lh + 1) * 512],
                                start=True,
                                stop=True,
                            )
                        if tt < NPOS:
                            pt = pospool.tile([128, Lq], BF16)
                            nc.sync.dma_start(
                                out=pt[:], in_=posT[h, tt * 128 : (tt + 1) * 128, :]
                            )
                            nc.vector.tensor_tensor(st[:], st[:], pt[:], op=ALU.add)
                        nc.scalar.activation(es[:, tq, :], st[:], AF.Exp)
                        if fillers:
                            fn, args = fillers.pop(0)
                            fn(*args)
                    for tq in range(NT // 2):
                        tt = half * (NT // 2) + tq
                        for lh in range(Lq // 512):
                            nc.tensor.matmul(
                                po[:, lh * 512 : (lh + 1) * 512],
                                lhsT=v_sb[:, tt, h * 65 : h * 65 + 65],
                                rhs=es[:, tq, lh * 512 : (lh + 1) * 512],
                                start=(tt == 0),
                                stop=(tt == NT - 1),
                            )
                # normalize: x^T_h = O'[0:64] * 1/Z
                z = small.tile([1, Lq], F32, tag="z")
                nc.vector.tensor_copy(z[:], po[64:65, :])
                r = small.tile([1, Lq], F32, tag="r")
                nc.vector.reciprocal_approx_fast(r[:], z[:])
                r64 = small.tile([64, Lq], F32, tag="r64")
                nc.gpsimd.partition_broadcast(r64[:], r[:])
                nc.vector.tensor_tensor(
                    xT_sb[lo:hi, p, :], po[0:64, :], r64[:], op=ALU.mult
                )

            # ---- output projection (partial): outp[o, l] ----
            for ot in range(C // 128):
                ob = osb.tile([128, Lq], F32)
                for lh in range(Lq // 512):
                    acc = ps.tile([128, 512], F32, tag="pj", name=f"o4_{ot}_{lh}")
                    for jc in range(JC // 128):
                        nc.tensor.matmul(
                            acc[:],
                            lhsT=pwT_sb[:, jc, ot * 128 : (ot + 1) * 128],
                            rhs=xT_sb[:, jc, lh * 512 : (lh + 1) * 512],
                            start=(jc == 0),
                            stop=(jc == JC // 128 - 1),
                        )
                    nc.vector.tensor_copy(ob[:, lh * 512 : (lh + 1) * 512], acc[:])
                nc.sync.dma_start(out=outp[ot * 128 : (ot + 1) * 128, :], in_=ob[:])

    nc.compile()
    return nc


_NC_CACHE = None


def _get_nc():
    global _NC_CACHE
    if _NC_CACHE is None:
        _NC_CACHE = build_kernel()
    return _NC_CACHE


def _prep_inputs(q, kv, attn_pos, q_w, kv_w, proj_w):
    bf = ml_dtypes.bfloat16
    qws = (q_w.astype(np.float64) * (D ** -0.5)).astype(np.float32)
    in_maps = []
    for c in range(N_CORES):
        b, g = c // 2, c % 2
        js = slice(g * JC, (g + 1) * JC)
        in_maps.append(
            {
                "qT": np.ascontiguousarray(q[b].T).astype(bf),
                "kvT": np.ascontiguousarray(kv[b].T).astype(bf),
                "qwT": np.ascontiguousarray(qws[js].T).astype(bf),
                "kwT": np.ascontiguousarray(kv_w[js].T).astype(bf),
                "vwT": np.ascontiguousarray(kv_w[C + g * JC : C + (g + 1) * JC].T).astype(bf),
                "posT": np.ascontiguousarray(
                    attn_pos[b, g * HPC : (g + 1) * HPC].transpose(0, 2, 1)
                ).astype(bf),
                "pwT": np.ascontiguousarray(proj_w[:, js].T).astype(bf),
            }
        )
    return in_maps


def kernel(q, kv, attn_pos, q_w, kv_w, proj_w, proj_b, _trace=False):
    q = np.asarray(q, dtype=np.float32)
    kv = np.asarray(kv, dtype=np.float32)
    attn_pos = np.asarray(attn_pos, dtype=np.float32)
    q_w = np.asarray(q_w, dtype=np.float32)
    kv_w = np.asarray(kv_w, dtype=np.float32)
    proj_w = np.asarray(proj_w, dtype=np.float32)
    proj_b = np.asarray(proj_b, dtype=np.float32)

    nc = _get_nc()
    in_maps = _prep_inputs(q, kv, attn_pos, q_w, kv_w, proj_w)
    res = run_bass_kernel_spmd(nc, in_maps, core_ids=list(range(N_CORES)), trace=_trace)
    kernel.last_results = res

    out = np.empty((B, Lq, C), np.float32)
    for b in range(B):
        part = res.results[2 * b]["outp"] + res.results[2 * b + 1]["outp"]
        out[b] = part.T + proj_b[None, :]
    return out


if __name__ == "__main__":
    rng = np.random.default_rng(0)
    ins = {
        "q": rng.standard_normal((B, Lq, C), np.float32),
        "kv": rng.standard_normal((B, Lkv, C), np.float32),
        "attn_pos": rng.standard_normal((B, H, Lq, Lpos), np.float32),
        "q_w": rng.standard_normal((C, C), np.float32) * 0.02,
        "kv_w": rng.standard_normal((2 * C, C), np.float32) * 0.02,
        "proj_w": rng.standard_normal((C, C), np.float32) * 0.02,
        "proj_b": np.zeros((C,), np.float32),
    }
    out = kernel(**ins)
    print("out", out.shape, out.dtype, float(np.abs(out).mean()))
